# revision 44
# baseline (speedup 1.0000x reference)
"""Multi-head causal attention (B=2, S=2048, D=1024, H=16) on 8 TRN2 NeuronCores.

Sharding: tensor-parallel over heads x data-parallel over batch.
Core c handles batch b = c // 4 and head group g = c % 4 (heads 4g..4g+3),
i.e. a [2048, 256] slice of the output.

v2 design (all fp32 data, matmuls in float32r):
  - The attention phase is ScalarE-bound (exp over all causal scores,
    ~58us of pure element time per core), so the kernel keeps the exp
    stream saturated and hides all other PE work inside it:
      * (hp, j, t) attention steps run serially with a 1-step score
        lookahead over the 2 sps PSUM buffers: per step, scores for step
        i+1 are issued before the PV matmuls of step i, so the PE never
        blocks the next exp.
      * All projection work that later steps need (V' tiles 4..15, Q/K
        n-blocks for head pair 1, and the NEXT loop iteration's V' tiles
        0..3 + first Q/K blocks) is issued as paced PE "filler" between
        attention steps, so it runs inside exp-wait gaps instead of
        serializing before/after the attention. In the hw repeat loop the
        filler tail of iteration r is the pre-phase of iteration r+1.
  - Normalization avoids ScalarE: 1/s via DVE reciprocal_approx_fast on
    the PV denominator row (the V' ones column), broadcast across 64
    partitions by a K=1 ones matmul into a spare PSUM slot one step
    later, then one DVE multiply (PSUM x PSUM -> SBUF) per head.
  - Scores computed transposed (S^T = K @ Q^T) per head pair into one
    [128, 1024] PSUM tile (two K=64 matmuls on distinct PE row groups via
    tile_position); one strided exp covers both heads.
  - PSUM budget: scores 2x[128,1024] (4 banks) + PV accum 1x[128,1024]
    (2 banks) + projection/V'/broadcast slots 2x[128,512] (2 banks) = 8.
  - Output written d-major [256, 2048], transposed on the host.
"""

import os
import sys

import numpy as np

for _p in ("/opt/trn_rl_repo", "/root/.axon_site/_ro/trn_rl_repo"):
    if os.path.isdir(_p) and _p not in sys.path:
        sys.path.insert(0, _p)

B, S, D, H = 2, 2048, 1024, 16
N_CORES = 8
HEADS_PER_CORE = 4
DH = D // H  # 64
DCORE = HEADS_PER_CORE * DH  # 256
KT = D // 128  # 8 contraction tiles for the projections
ST = S // 128  # 16 sequence tiles
QB = 512  # q block width
NEG = -1.0e30

_CACHE = {}


def _split_multi_waits(nc, max_waits=1):
    """This walrus build rejects instructions carrying more than one
    semaphore wait; hoist extras onto preceding NoOps on the same engine."""
    import bass_rust as _br

    n = 0
    for fn in nc.m.functions:
        for bb in fn.blocks:
            insts = list(bb.instructions)
            new = []
            changed = False
            for inst in insts:
                si = getattr(inst, "sync_info", None)
                ow = list(si.on_wait) if si is not None else []
                if len(ow) > max_waits:
                    changed = True
                    for w in ow[:-max_waits]:
                        n += 1
                        new.append(
                            _br.InstNoOp(
                                name=f"I-ws{n}",
                                engine=inst.engine,
                                ins=[],
                                outs=[],
                                sync_info=_br.SyncInfo(on_wait=[w], on_update=[]),
                            )
                        )
                    si.on_wait = ow[-max_waits:]
                    inst.sync_info = si
                new.append(inst)
            if changed:
                bb.instructions = new


def build_module(repeat=1, hw_loop=False, split_waits=True):
    import contextlib

    import concourse.bass as bass
    import concourse.library_config as library_config
    import concourse.mybir as mybir
    from concourse.tile import TileContext

    F32 = mybir.dt.float32
    F32R = mybir.dt.float32r
    AF = mybir.ActivationFunctionType

    nc = bass.Bass("TRN2", target_bir_lowering=False, debug=False, num_devices=N_CORES)

    xT_in = nc.declare_dram_parameter("xT", [D, S], F32, isOutput=False)
    wq_in = nc.declare_dram_parameter("wq", [D, DCORE], F32, isOutput=False)
    wk_in = nc.declare_dram_parameter("wk", [D, DCORE], F32, isOutput=False)
    wv_in = nc.declare_dram_parameter("wv", [D, DCORE], F32, isOutput=False)
    bq_in = nc.declare_dram_parameter("bq", [DCORE], F32, isOutput=False)
    bk_in = nc.declare_dram_parameter("bk", [DCORE], F32, isOutput=False)
    bv_in = nc.declare_dram_parameter("bv", [DCORE], F32, isOutput=False)
    tri_in = nc.declare_dram_parameter("tri", [128, 256], F32, isOutput=False)
    tri2_in = nc.declare_dram_parameter("tri2", [128, 512], F32, isOutput=False)
    ones_in = nc.declare_dram_parameter("ones", [128, 4], F32, isOutput=False)
    outT = nc.declare_dram_parameter("outT", [DCORE, S], F32, isOutput=True)

    with TileContext(nc) as tc:
        with (
            tc.tile_pool(name="persist", bufs=1) as pp,
            tc.tile_pool(name="work", bufs=4) as wp,
            tc.tile_pool(name="norm", bufs=2) as nrm,
            tc.tile_pool(name="outp", bufs=3) as op,
            tc.tile_pool(name="sc_ps", bufs=2, space="PSUM") as sc_ps,
            tc.tile_pool(name="ap_ps", bufs=1, space="PSUM") as ap_ps,
            tc.tile_pool(name="qk_ps", bufs=2, space="PSUM") as qk_ps,
        ):
            # ---- constant / persistent tiles -------------------------------
            trid = pp.tile([128, 256], F32, tag="trid")
            nc.sync.dma_start(trid[:], tri_in[:])
            # i==3 mask: [full -1e30 block | triangle] per head half, so the
            # diagonal window can pad to 256 wide (fp32r needs a moving dim
            # >= 256 for 1 cycle/row; 128-wide runs at 4)
            trid2 = pp.tile([128, 512], F32, tag="trid2")
            nc.sync.dma_start(trid2[:], tri2_in[:])
            onesr = pp.tile([1, 128], F32R, tag="onesr")  # K=1 matmul lhsT
            nc.sync.dma_start(
                onesr[:], ones_in[:, 0:1].rearrange("p a -> a p").bitcast(F32R)
            )
            bvrow = pp.tile([1, DCORE], F32R, tag="bvrow")
            nc.sync.dma_start(
                bvrow[:], bv_in[:].rearrange("(a b) -> a b", a=1).bitcast(F32R)
            )
            bqc = pp.tile([128, 2], F32, tag="bqc")
            nc.sync.dma_start(bqc[:], bq_in[:].rearrange("(m p) -> p m", p=128))
            bkc = pp.tile([128, 2], F32, tag="bkc")
            nc.sync.dma_start(bkc[:], bk_in[:].rearrange("(m p) -> p m", p=128))

            wq = []
            wk = []
            wv = []
            for k in range(KT):
                for name, lst, src in (("wq", wq, wq_in), ("wk", wk, wk_in), ("wv", wv, wv_in)):
                    t = pp.tile([128, DCORE], F32R, tag=f"{name}{k}")
                    nc.sync.dma_start(
                        t[:], src[128 * k : 128 * (k + 1), :].bitcast(F32R)
                    )
                    lst.append(t)

            # persistent activation tiles
            qT = [pp.tile([128, S], F32R, tag=f"qT{m}", name=f"qT{m}") for m in range(2)]
            kTt = [pp.tile([128, S], F32R, tag=f"kT{m}", name=f"kT{m}") for m in range(2)]
            vp = [pp.tile([128, 4 * 65], F32R, tag=f"vp{s}", name=f"vp{s}") for s in range(ST)]
            # ones columns of V' written once; V' copies only touch [:, :, 0:64]
            for s in range(ST):
                nc.sync.dma_start(
                    vp[s][:].rearrange("p (h c) -> p h c", c=65)[:, :, 64:65],
                    ones_in[:].rearrange("p (h c) -> p h c", c=1).bitcast(F32R),
                )

            # ---- warmup during the x DMA window: ~4.5us of dummy matmuls
            # ramps the PE HAM clock gate to 2.4 GHz, and one exp pulls the
            # activation table load off the critical path -------------------
            warm_ps = qk_ps.tile([128, QB], F32, tag="qk", name="warm_ps")
            for _w in range(42):
                nc.tensor.matmul(
                    warm_ps[:, 0:DCORE], onesr[:], bvrow[:], start=True, stop=True
                )
            warm_o = wp.tile([1, 128], F32, tag="warm", name="warm_o")
            nc.scalar.activation(warm_o[:], onesr[:].bitcast(F32), AF.Exp)
            nc.scalar.activation(warm_o[:], warm_o[:], AF.Ln)

            # x^T tiles, loaded in [128, QB] slices n-major so the first
            # projection blocks can start after ~1/4 of x has landed
            xt = [pp.tile([128, S], F32R, tag=f"xt{k}", name=f"xt{k}") for k in range(KT)]
            for n in range(S // QB):
                for k in range(KT):
                    nc.sync.dma_start(
                        xt[k][:, QB * n : QB * (n + 1)],
                        xT_in[128 * k : 128 * (k + 1), QB * n : QB * (n + 1)].bitcast(
                            F32R
                        ),
                    )

            # ---------------- helper op builders ---------------------------
            def v_tile(s):
                """Project V' for sequence tile s into vp[s]."""
                ps = qk_ps.tile([128, DCORE], F32, tag="qk", name=f"vps{s}")
                for k in range(KT):
                    nc.tensor.matmul(
                        ps[:],
                        xt[k][:, 128 * s : 128 * (s + 1)],
                        wv[k][:],
                        start=(k == 0),
                        stop=False,
                    )
                nc.tensor.matmul(ps[:], onesr[:], bvrow[:], start=False, stop=True)
                nc.vector.tensor_copy(
                    vp[s][:].rearrange("p (h c) -> p h c", c=65)[:, :, 0:64],
                    ps[:].rearrange("p (h c) -> p h c", c=64),
                )

            def qk_block(which, m, n):
                """Project one [128, QB] n-block of Q^T or K^T (head pair m)."""
                w, bias, dst = (wq, bqc, qT) if which == "q" else (wk, bkc, kTt)
                acc = qk_ps.tile([128, QB], F32, tag="qk", name=f"{which}{m}n{n}")
                for k in range(KT):
                    nc.tensor.matmul(
                        acc[:],
                        w[k][:, 128 * m : 128 * (m + 1)],
                        xt[k][:, QB * n : QB * (n + 1)],
                        start=(k == 0),
                        stop=(k == KT - 1),
                    )
                nc.vector.tensor_scalar_add(
                    dst[m][:, QB * n : QB * (n + 1)], acc[:], bias[:, m : m + 1]
                )

            def prephase():
                for s in range(4):
                    v_tile(s)
                qk_block("q", 0, 0)
                qk_block("k", 0, 0)

            prephase()

            if hw_loop and repeat > 1:
                rep_iter = [0]
                rep_ctx = tc.For_i(0, repeat, 1)
            else:
                rep_iter = range(repeat)
                rep_ctx = contextlib.nullcontext()
            with rep_ctx:
              for _rep in rep_iter:
                # ---- filler queue: PE work issued between attention steps.
                # Each item carries (ready, deadline) step indices: deadline
                # guarantees issue before its first consumer (the j-start
                # scores read Q/K n-blocks, PV reads V' tiles); ready keeps
                # next-iteration rewrites from stalling the DVE queue behind
                # a WAR wait. The tail recomputes the NEXT iteration's
                # pre-phase (V' 0..3 + first Q/K blocks of head pair 0).
                j_start = {0: 0, 1: 4, 2: 12, 3: 24}
                filler = []
                for nblk in range(1, 4):
                    filler.append((0, j_start[nblk] - 2, ("qk", "q", 0, nblk)))
                    filler.append((0, j_start[nblk] - 2, ("qk", "k", 0, nblk)))
                for s in range(4, ST):
                    filler.append((0, j_start[s // 4] + s - 1, ("v", s)))
                for nblk in range(4):
                    filler.append((0, 40 + j_start[nblk] - 2, ("qk", "q", 1, nblk)))
                    filler.append((0, 40 + j_start[nblk] - 2, ("qk", "k", 1, nblk)))
                # next-iteration pre-phase, ready-gated so rewrites can't
                # stall the DVE queue behind WAR waits
                for s in range(4):
                    filler.append((66 + s, 78, ("v", s)))
                filler.append((41, 79, ("qk", "q", 0, 0)))
                filler.append((41, 79, ("qk", "k", 0, 0)))
                filler.sort(key=lambda it: it[1])
                n_filler = len(filler)
                popped = [0]

                def do_item(item):
                    popped[0] += 1
                    if item[0] == "v":
                        v_tile(item[1])
                    else:
                        qk_block(item[1], item[2], item[3])

                def pop_filler(idx, bonus):
                    # deadline-due items first
                    while filler and filler[0][1] <= idx:
                        do_item(filler.pop(0)[2])
                    # gentle proportional pacing + boundary bonus; skip over
                    # items whose ready step hasn't arrived yet
                    want = max(
                        (idx + 1) * n_filler // n_steps - popped[0], bonus
                    )
                    while want > 0:
                        pick = next(
                            (i for i, it in enumerate(filler) if it[0] <= idx), None
                        )
                        if pick is None:
                            break
                        do_item(filler.pop(pick)[2])
                        want -= 1

                # ---- attention steps: (hp, j, t) serial, 1-step score
                # lookahead; PV of step i issued after scores of step i+1 ----
                steps = [
                    (hp, j, t)
                    for hp in range(2)
                    for j in range(4)
                    for t in range(4 * j + 4)
                ]
                n_steps = len(steps)

                pts = {}

                def issue_scores(hp, j, t):
                    i = t - 4 * j  # >= 0 on diagonal-region tiles
                    qoff = 128 * max(i, 0) if i != 3 else 256
                    qwin = slice(QB * j + qoff, QB * (j + 1))
                    ktile = slice(128 * t, 128 * (t + 1))
                    qTm, kTm = qT[hp], kTt[hp]
                    sps = sc_ps.tile([128, 2 * QB], F32, tag="mm", name="sps")
                    nc.tensor.matmul(
                        sps[:, qoff:QB],
                        kTm[0:64, ktile],
                        qTm[0:64, qwin],
                        start=True,
                        stop=True,
                        tile_position=(0, 0),
                    )
                    nc.tensor.matmul(
                        sps[:, QB + qoff : 2 * QB],
                        kTm[64:128, ktile],
                        qTm[64:128, qwin],
                        start=True,
                        stop=True,
                        tile_position=(64, 0),
                    )
                    spsv = sps[:].rearrange("p (two c) -> p two c", two=2)
                    if 0 <= i < 3:
                        nc.vector.tensor_add(
                            spsv[:, :, qoff : qoff + 128],
                            spsv[:, :, qoff : qoff + 128],
                            trid[:].rearrange("p (two c) -> p two c", two=2),
                        )
                    elif i == 3:
                        nc.vector.tensor_add(
                            spsv[:, :, qoff : qoff + 256],
                            spsv[:, :, qoff : qoff + 256],
                            trid2[:].rearrange("p (two c) -> p two c", two=2),
                        )
                    pt = wp.tile([128, 2 * QB], F32R, tag="pt")
                    ptv = pt[:].rearrange("p (two c) -> p two c", two=2)
                    nc.scalar.activation(
                        ptv[:, :, qoff:QB],
                        spsv[:, :, qoff:QB],
                        AF.Exp,
                        scale=float(1.0 / np.sqrt(DH)),
                    )
                    pts[(hp, j, t)] = (pt, qoff)

                aps_cur = [None]

                def issue_pv(hp, j, t):
                    if t == 0:
                        aps_cur[0] = ap_ps.tile(
                            [128, 2 * QB], F32, tag="att", name=f"aps{hp}_{j}"
                        )
                    ap = aps_cur[0]
                    pt, qoff = pts.pop((hp, j, t))
                    hA, hB = 2 * hp, 2 * hp + 1
                    for h, off in ((hA, 0), (hB, QB)):
                        nc.tensor.matmul(
                            ap[0:65, off + qoff : off + QB],
                            vp[t][:, 65 * h : 65 * h + 65],
                            pt[:, off + qoff : off + QB],
                            start=(t == 0),
                            stop=(t == 4 * j + 3),
                        )

                # Normalization is staged across attention steps so no engine
                # waits inline: stage 1 (at j's last step): DVE
                # reciprocal_approx_fast of the PV denominator row; stage 2
                # (one step later): K=1 ones-matmul broadcast into qk_ps
                # slots + DVE copy to SBUF (DVE has a single PSUM port, so
                # the mul cannot read two PSUM operands); stage 3: DVE mul
                # (PSUM x SBUF -> SBUF) + output DMA. The PV accumulator
                # frees when stage-3's mul completes.
                # Normalization: stage 1 snapshot-copies the PV accumulator
                # to SBUF (one DVE op — this alone frees the accumulator's
                # PSUM banks, so the next j-block's PV never waits on the
                # rest of the chain). Then 1/s = exp(-ln(s)) on ScalarE
                # (custom DVE reciprocal ops don't compile on this walrus
                # build; plain DVE reciprocal runs ~7 cycles/element;
                # AF.Reciprocal lives in a different activation-table set
                # than Exp, and a set switch is ~2.7us), broadcast by K=1
                # ones matmuls, copied to SBUF, multiplied on DVE.
                def norm_stage1(hp, j, ap):
                    cp = nrm.tile([65, 2 * QB], F32, tag="cp", name="cp")
                    nc.vector.tensor_copy(cp[:], ap[0:65, :])
                    return cp

                def norm_stage2(cp):
                    lns = nrm.tile([1, 2 * QB], F32, tag="lns", name="lns")
                    nc.scalar.activation(lns[:], cp[64:65, :], AF.Ln)
                    return lns

                def norm_stage3(cp, lns):
                    # ScalarE writes the F32R rounding the broadcast needs
                    rrr = nrm.tile([1, 2 * QB], F32R, tag="rrr", name="rrr")
                    nc.scalar.activation(rrr[:], lns[:], AF.Exp, scale=-1.0)
                    rb = nrm.tile([64, 2 * QB], F32, tag="rb", name="rb")
                    for off in (0, QB):
                        rbp = qk_ps.tile([64, QB], F32, tag="qk", name="rbp")
                        nc.tensor.matmul(
                            rbp[:],
                            onesr[:, 0:64],
                            rrr[:, off : off + QB],
                            start=True,
                            stop=True,
                        )
                        nc.vector.tensor_copy(rb[:, off : off + QB], rbp[:])
                    return rb

                def norm_stage4(hp, j, cp, rb):
                    # DVE multiply: GpSimd tensor ops measured ~35us slower
                    # per iteration on hardware than CoreSim's cost model
                    att = op.tile([64, 2 * QB], F32, tag="att_out", name="att")
                    nc.vector.tensor_mul(att[:], cp[0:64, :], rb[:])
                    hA, hB = 2 * hp, 2 * hp + 1
                    for h, off in ((hA, 0), (hB, QB)):
                        nc.sync.dma_start(
                            outT[64 * h : 64 * (h + 1), QB * j : QB * (j + 1)],
                            att[:, off : off + QB],
                        )

                norm_q = []  # (stage, hp, j, cp, aux)

                def advance_norm():
                    if not norm_q:
                        return
                    stage, hp, j, cp, aux = norm_q.pop(0)
                    if stage == 2:
                        norm_q.append((3, hp, j, cp, norm_stage2(cp)))
                    elif stage == 3:
                        norm_q.append((4, hp, j, cp, norm_stage3(cp, aux)))
                    else:
                        norm_stage4(hp, j, cp, aux)

                issue_scores(*steps[0])
                for idx, (hp, j, t) in enumerate(steps):
                    if idx + 1 < n_steps:
                        issue_scores(*steps[idx + 1])
                    # extra fillers at j starts, where the new PV accumulator
                    # waits on the previous j's normalization drain
                    boundary = t == 4 * j + 3
                    pop_filler(idx, 2 if t == 0 else 0)
                    advance_norm()
                    issue_pv(hp, j, t)
                    if boundary:
                        cp = norm_stage1(hp, j, aps_cur[0])
                        norm_q.append((2, hp, j, cp, None))
                while norm_q:
                    advance_norm()
                while filler:
                    do_item(filler.pop(0)[2])

    if split_waits:
        _split_multi_waits(nc)
    return nc


def _get_runner():
    if "nc" not in _CACHE:
        _CACHE["nc"] = build_module()
    return _CACHE["nc"]


def _make_in_maps(x, Wq, bq, Wk, bk, Wv, bv):
    x = np.asarray(x, dtype=np.float32)
    Wq = np.asarray(Wq, dtype=np.float32)
    Wk = np.asarray(Wk, dtype=np.float32)
    Wv = np.asarray(Wv, dtype=np.float32)
    bq = np.asarray(bq, dtype=np.float32)
    bk = np.asarray(bk, dtype=np.float32)
    bv = np.asarray(bv, dtype=np.float32)

    kp = np.arange(128)[:, None]
    qf = np.arange(128)[None, :]
    tri = np.where(kp <= qf, 0.0, NEG).astype(np.float32)
    trid = np.concatenate([tri, tri], axis=1)
    full = np.full((128, 128), NEG, np.float32)
    tri2 = np.concatenate([full, tri, full, tri], axis=1)
    ones = np.ones((128, 4), np.float32)

    xTs = [np.ascontiguousarray(x[b].T) for b in range(B)]
    in_maps = []
    for c in range(N_CORES):
        b = c // 4
        g = c % 4
        sl = slice(DCORE * g, DCORE * (g + 1))
        in_maps.append(
            {
                "xT": xTs[b],
                "wq": np.ascontiguousarray(Wq[:, sl]),
                "wk": np.ascontiguousarray(Wk[:, sl]),
                "wv": np.ascontiguousarray(Wv[:, sl]),
                "bq": np.ascontiguousarray(bq[sl]),
                "bk": np.ascontiguousarray(bk[sl]),
                "bv": np.ascontiguousarray(bv[sl]),
                "tri": trid,
                "tri2": tri2,
                "ones": ones,
            }
        )
    return in_maps


def kernel(x, Wq, bq, Wk, bk, Wv, bv):
    from concourse.bass_utils import run_bass_kernel_spmd

    nc = _get_runner()
    in_maps = _make_in_maps(x, Wq, bq, Wk, bk, Wv, bv)
    res = run_bass_kernel_spmd(nc, in_maps, list(range(N_CORES)))
    out = np.empty((B, S, D), dtype=np.float32)
    for c in range(N_CORES):
        b = c // 4
        g = c % 4
        out[b, :, DCORE * g : DCORE * (g + 1)] = res.results[c]["outT"].T
    return out


# revision 45
# speedup vs baseline: 1.0010x; 1.0010x over previous
"""Multi-head causal attention (B=2, S=2048, D=1024, H=16) on 8 TRN2 NeuronCores.

Sharding: tensor-parallel over heads x data-parallel over batch.
Core c handles batch b = c // 4 and head group g = c % 4 (heads 4g..4g+3),
i.e. a [2048, 256] slice of the output.

v2 design (all fp32 data, matmuls in float32r; fp8 was evaluated and
rejected — e4m3 quantization of any of Q/K/V alone already exceeds the
2e-2 absmax gate on softmax-concentrated rows):
  - Attention runs as 80 serial (hp, j, t) steps with a 1-step score
    lookahead over the 2 sps PSUM buffers: per step, scores for step i+1
    are issued before the PV matmuls of step i, so the PE rarely blocks
    on the exp stream and ScalarE stays saturated.
  - All projection work (V' tiles 4..15, Q/K n-blocks for head pair 1,
    and the NEXT loop iteration's V' 0..3 + first Q/K blocks) is issued
    as deadline-paced PE "filler" between attention steps, so it runs
    inside exp-wait gaps instead of serializing before/after attention.
    In the hw repeat loop the filler tail of iteration r is the
    pre-phase of iteration r+1 — steady-state has no serial phase.
  - Scores computed transposed (S^T = K @ Q^T) per head pair into one
    [128, 1024] PSUM tile (two K=64 matmuls on distinct PE row groups
    via tile_position); one strided exp covers both heads. Diagonal
    windows pad to 256 wide (fp32r moving <256 runs 4 cycles/row), with
    a [full -1e30 | triangle] mask for the padded block.
  - Normalization is staged over 4 attention steps so nothing inline
    waits: (1) DVE snapshot-copy of the PV accumulator to SBUF — this
    alone frees the accumulator PSUM banks for the next j-block; (2)
    ScalarE ln; (3) ScalarE exp(-x) + K=1 ones-matmul broadcast + DVE
    copies to SBUF; (4) DVE multiply + output DMA. (Custom-DVE
    reciprocal ops and GpSimd ops fail/underperform on this toolchain.)
  - PSUM budget: scores 2x[128,1024] (4 banks) + PV accum 1x[128,1024]
    (2 banks) + projection/V'/broadcast slots 2x[128,512] (2 banks) = 8.
  - Output written d-major [256, 2048], transposed on the host.
"""

import os
import sys

import numpy as np

for _p in ("/opt/trn_rl_repo", "/root/.axon_site/_ro/trn_rl_repo"):
    if os.path.isdir(_p) and _p not in sys.path:
        sys.path.insert(0, _p)

B, S, D, H = 2, 2048, 1024, 16
N_CORES = 8
HEADS_PER_CORE = 4
DH = D // H  # 64
DCORE = HEADS_PER_CORE * DH  # 256
KT = D // 128  # 8 contraction tiles for the projections
ST = S // 128  # 16 sequence tiles
QB = 512  # q block width
NEG = -1.0e30

_CACHE = {}


def _split_multi_waits(nc, max_waits=1):
    """This walrus build rejects instructions carrying more than one
    semaphore wait; hoist extras onto preceding NoOps on the same engine."""
    import bass_rust as _br

    n = 0
    for fn in nc.m.functions:
        for bb in fn.blocks:
            insts = list(bb.instructions)
            new = []
            changed = False
            for inst in insts:
                si = getattr(inst, "sync_info", None)
                ow = list(si.on_wait) if si is not None else []
                if len(ow) > max_waits:
                    changed = True
                    for w in ow[:-max_waits]:
                        n += 1
                        new.append(
                            _br.InstNoOp(
                                name=f"I-ws{n}",
                                engine=inst.engine,
                                ins=[],
                                outs=[],
                                sync_info=_br.SyncInfo(on_wait=[w], on_update=[]),
                            )
                        )
                    si.on_wait = ow[-max_waits:]
                    inst.sync_info = si
                new.append(inst)
            if changed:
                bb.instructions = new


def build_module(repeat=1, hw_loop=False, split_waits=True):
    import contextlib

    import concourse.bass as bass
    import concourse.library_config as library_config
    import concourse.mybir as mybir
    from concourse.tile import TileContext

    F32 = mybir.dt.float32
    F32R = mybir.dt.float32r
    AF = mybir.ActivationFunctionType

    nc = bass.Bass("TRN2", target_bir_lowering=False, debug=False, num_devices=N_CORES)

    xT_in = nc.declare_dram_parameter("xT", [D, S], F32, isOutput=False)
    wq_in = nc.declare_dram_parameter("wq", [D, DCORE], F32, isOutput=False)
    wk_in = nc.declare_dram_parameter("wk", [D, DCORE], F32, isOutput=False)
    wv_in = nc.declare_dram_parameter("wv", [D, DCORE], F32, isOutput=False)
    bq_in = nc.declare_dram_parameter("bq", [DCORE], F32, isOutput=False)
    bk_in = nc.declare_dram_parameter("bk", [DCORE], F32, isOutput=False)
    bv_in = nc.declare_dram_parameter("bv", [DCORE], F32, isOutput=False)
    tri_in = nc.declare_dram_parameter("tri", [128, 256], F32, isOutput=False)
    tri2_in = nc.declare_dram_parameter("tri2", [128, 512], F32, isOutput=False)
    ones_in = nc.declare_dram_parameter("ones", [128, 4], F32, isOutput=False)
    outT = nc.declare_dram_parameter("outT", [DCORE, S], F32, isOutput=True)

    with TileContext(nc) as tc:
        with (
            tc.tile_pool(name="persist", bufs=1) as pp,
            tc.tile_pool(name="work", bufs=4) as wp,
            tc.tile_pool(name="norm", bufs=2) as nrm,
            tc.tile_pool(name="outp", bufs=3) as op,
            tc.tile_pool(name="sc_ps", bufs=2, space="PSUM") as sc_ps,
            tc.tile_pool(name="ap_ps", bufs=1, space="PSUM") as ap_ps,
            tc.tile_pool(name="qk_ps", bufs=2, space="PSUM") as qk_ps,
        ):
            # ---- constant / persistent tiles -------------------------------
            trid = pp.tile([128, 256], F32, tag="trid")
            nc.sync.dma_start(trid[:], tri_in[:])
            # i==3 mask: [full -1e30 block | triangle] per head half, so the
            # diagonal window can pad to 256 wide (fp32r needs a moving dim
            # >= 256 for 1 cycle/row; 128-wide runs at 4)
            trid2 = pp.tile([128, 512], F32, tag="trid2")
            nc.sync.dma_start(trid2[:], tri2_in[:])
            onesr = pp.tile([1, 128], F32R, tag="onesr")  # K=1 matmul lhsT
            nc.sync.dma_start(
                onesr[:], ones_in[:, 0:1].rearrange("p a -> a p").bitcast(F32R)
            )
            bvrow = pp.tile([1, DCORE], F32R, tag="bvrow")
            nc.sync.dma_start(
                bvrow[:], bv_in[:].rearrange("(a b) -> a b", a=1).bitcast(F32R)
            )
            bqc = pp.tile([128, 2], F32, tag="bqc")
            nc.sync.dma_start(bqc[:], bq_in[:].rearrange("(m p) -> p m", p=128))
            bkc = pp.tile([128, 2], F32, tag="bkc")
            nc.sync.dma_start(bkc[:], bk_in[:].rearrange("(m p) -> p m", p=128))

            wq = []
            wk = []
            wv = []
            for k in range(KT):
                for name, lst, src in (("wq", wq, wq_in), ("wk", wk, wk_in), ("wv", wv, wv_in)):
                    t = pp.tile([128, DCORE], F32R, tag=f"{name}{k}")
                    nc.sync.dma_start(
                        t[:], src[128 * k : 128 * (k + 1), :].bitcast(F32R)
                    )
                    lst.append(t)

            # persistent activation tiles
            qT = [pp.tile([128, S], F32R, tag=f"qT{m}", name=f"qT{m}") for m in range(2)]
            kTt = [pp.tile([128, S], F32R, tag=f"kT{m}", name=f"kT{m}") for m in range(2)]
            vp = [pp.tile([128, 4 * 65], F32R, tag=f"vp{s}", name=f"vp{s}") for s in range(ST)]
            # ones columns of V' written once; V' copies only touch [:, :, 0:64]
            for s in range(ST):
                nc.sync.dma_start(
                    vp[s][:].rearrange("p (h c) -> p h c", c=65)[:, :, 64:65],
                    ones_in[:].rearrange("p (h c) -> p h c", c=1).bitcast(F32R),
                )

            # ---- warmup during the x DMA window: ~4.5us of dummy matmuls
            # ramps the PE HAM clock gate to 2.4 GHz, and one exp pulls the
            # activation table load off the critical path -------------------
            warm_ps = qk_ps.tile([128, QB], F32, tag="qk", name="warm_ps")
            for _w in range(42):
                nc.tensor.matmul(
                    warm_ps[:, 0:DCORE], onesr[:], bvrow[:], start=True, stop=True
                )
            warm_o = wp.tile([1, 128], F32, tag="warm", name="warm_o")
            nc.scalar.activation(warm_o[:], onesr[:].bitcast(F32), AF.Exp)
            nc.scalar.activation(warm_o[:], warm_o[:], AF.Ln)

            # x^T tiles, loaded in [128, QB] slices n-major so the first
            # projection blocks can start after ~1/4 of x has landed
            xt = [pp.tile([128, S], F32R, tag=f"xt{k}", name=f"xt{k}") for k in range(KT)]
            for n in range(S // QB):
                for k in range(KT):
                    nc.sync.dma_start(
                        xt[k][:, QB * n : QB * (n + 1)],
                        xT_in[128 * k : 128 * (k + 1), QB * n : QB * (n + 1)].bitcast(
                            F32R
                        ),
                    )

            # ---------------- helper op builders ---------------------------
            def v_tile(s):
                """Project V' for sequence tile s into vp[s]."""
                ps = qk_ps.tile([128, DCORE], F32, tag="qk", name=f"vps{s}")
                for k in range(KT):
                    nc.tensor.matmul(
                        ps[:],
                        xt[k][:, 128 * s : 128 * (s + 1)],
                        wv[k][:],
                        start=(k == 0),
                        stop=False,
                    )
                nc.tensor.matmul(ps[:], onesr[:], bvrow[:], start=False, stop=True)
                nc.vector.tensor_copy(
                    vp[s][:].rearrange("p (h c) -> p h c", c=65)[:, :, 0:64],
                    ps[:].rearrange("p (h c) -> p h c", c=64),
                )

            def qk_block(which, m, n):
                """Project one [128, QB] n-block of Q^T or K^T (head pair m)."""
                w, bias, dst = (wq, bqc, qT) if which == "q" else (wk, bkc, kTt)
                acc = qk_ps.tile([128, QB], F32, tag="qk", name=f"{which}{m}n{n}")
                for k in range(KT):
                    nc.tensor.matmul(
                        acc[:],
                        w[k][:, 128 * m : 128 * (m + 1)],
                        xt[k][:, QB * n : QB * (n + 1)],
                        start=(k == 0),
                        stop=(k == KT - 1),
                    )
                nc.vector.tensor_scalar_add(
                    dst[m][:, QB * n : QB * (n + 1)], acc[:], bias[:, m : m + 1]
                )

            def prephase():
                for s in range(4):
                    v_tile(s)
                qk_block("q", 0, 0)
                qk_block("k", 0, 0)

            prephase()

            if hw_loop and repeat > 1:
                rep_iter = [0]
                rep_ctx = tc.For_i(0, repeat, 1)
            else:
                rep_iter = range(repeat)
                rep_ctx = contextlib.nullcontext()
            with rep_ctx:
              for _rep in rep_iter:
                # ---- filler queue: PE work issued between attention steps.
                # Each item carries (ready, deadline) step indices: deadline
                # guarantees issue before its first consumer (the j-start
                # scores read Q/K n-blocks, PV reads V' tiles); ready keeps
                # next-iteration rewrites from stalling the DVE queue behind
                # a WAR wait. The tail recomputes the NEXT iteration's
                # pre-phase (V' 0..3 + first Q/K blocks of head pair 0).
                j_start = {0: 0, 1: 4, 2: 12, 3: 24}
                filler = []
                for nblk in range(1, 4):
                    filler.append((0, j_start[nblk] - 2, ("qk", "q", 0, nblk)))
                    filler.append((0, j_start[nblk] - 2, ("qk", "k", 0, nblk)))
                for s in range(4, ST):
                    filler.append((0, j_start[s // 4] + s - 1, ("v", s)))
                for nblk in range(4):
                    filler.append((0, 40 + j_start[nblk] - 2, ("qk", "q", 1, nblk)))
                    filler.append((0, 40 + j_start[nblk] - 2, ("qk", "k", 1, nblk)))
                # next-iteration pre-phase, ready-gated so rewrites can't
                # stall the DVE queue behind WAR waits
                for s in range(4):
                    filler.append((66 + s, 78, ("v", s)))
                filler.append((41, 79, ("qk", "q", 0, 0)))
                filler.append((41, 79, ("qk", "k", 0, 0)))
                filler.sort(key=lambda it: it[1])
                n_filler = len(filler)
                popped = [0]

                def do_item(item):
                    popped[0] += 1
                    if item[0] == "v":
                        v_tile(item[1])
                    else:
                        qk_block(item[1], item[2], item[3])

                def pop_filler(idx, bonus):
                    # deadline-due items first
                    while filler and filler[0][1] <= idx:
                        do_item(filler.pop(0)[2])
                    # gentle proportional pacing + boundary bonus; skip over
                    # items whose ready step hasn't arrived yet
                    want = max(
                        (idx + 1) * n_filler // n_steps - popped[0], bonus
                    )
                    while want > 0:
                        pick = next(
                            (i for i, it in enumerate(filler) if it[0] <= idx), None
                        )
                        if pick is None:
                            break
                        do_item(filler.pop(pick)[2])
                        want -= 1

                # ---- attention steps: (hp, j, t) serial, 1-step score
                # lookahead; PV of step i issued after scores of step i+1 ----
                steps = [
                    (hp, j, t)
                    for hp in range(2)
                    for j in range(4)
                    for t in range(4 * j + 4)
                ]
                n_steps = len(steps)

                pts = {}

                def issue_scores(hp, j, t):
                    i = t - 4 * j  # >= 0 on diagonal-region tiles
                    qoff = 128 * max(i, 0) if i != 3 else 256
                    qwin = slice(QB * j + qoff, QB * (j + 1))
                    ktile = slice(128 * t, 128 * (t + 1))
                    qTm, kTm = qT[hp], kTt[hp]
                    sps = sc_ps.tile([128, 2 * QB], F32, tag="mm", name="sps")
                    nc.tensor.matmul(
                        sps[:, qoff:QB],
                        kTm[0:64, ktile],
                        qTm[0:64, qwin],
                        start=True,
                        stop=True,
                        tile_position=(0, 0),
                    )
                    nc.tensor.matmul(
                        sps[:, QB + qoff : 2 * QB],
                        kTm[64:128, ktile],
                        qTm[64:128, qwin],
                        start=True,
                        stop=True,
                        tile_position=(64, 0),
                    )
                    spsv = sps[:].rearrange("p (two c) -> p two c", two=2)
                    if 0 <= i < 3:
                        nc.vector.tensor_add(
                            spsv[:, :, qoff : qoff + 128],
                            spsv[:, :, qoff : qoff + 128],
                            trid[:].rearrange("p (two c) -> p two c", two=2),
                        )
                    elif i == 3:
                        nc.vector.tensor_add(
                            spsv[:, :, qoff : qoff + 256],
                            spsv[:, :, qoff : qoff + 256],
                            trid2[:].rearrange("p (two c) -> p two c", two=2),
                        )
                    pt = wp.tile([128, 2 * QB], F32R, tag="pt")
                    ptv = pt[:].rearrange("p (two c) -> p two c", two=2)
                    nc.scalar.activation(
                        ptv[:, :, qoff:QB],
                        spsv[:, :, qoff:QB],
                        AF.Exp,
                        scale=float(1.0 / np.sqrt(DH)),
                    )
                    pts[(hp, j, t)] = (pt, qoff)

                aps_cur = [None]

                def issue_pv(hp, j, t):
                    if t == 0:
                        aps_cur[0] = ap_ps.tile(
                            [128, 2 * QB], F32, tag="att", name=f"aps{hp}_{j}"
                        )
                    ap = aps_cur[0]
                    pt, qoff = pts.pop((hp, j, t))
                    hA, hB = 2 * hp, 2 * hp + 1
                    for h, off in ((hA, 0), (hB, QB)):
                        nc.tensor.matmul(
                            ap[0:65, off + qoff : off + QB],
                            vp[t][:, 65 * h : 65 * h + 65],
                            pt[:, off + qoff : off + QB],
                            start=(t == 0),
                            stop=(t == 4 * j + 3),
                        )

                # Normalization is staged across attention steps so no engine
                # waits inline: stage 1 (at j's last step): DVE
                # reciprocal_approx_fast of the PV denominator row; stage 2
                # (one step later): K=1 ones-matmul broadcast into qk_ps
                # slots + DVE copy to SBUF (DVE has a single PSUM port, so
                # the mul cannot read two PSUM operands); stage 3: DVE mul
                # (PSUM x SBUF -> SBUF) + output DMA. The PV accumulator
                # frees when stage-3's mul completes.
                # Normalization: stage 1 snapshot-copies the PV accumulator
                # to SBUF (one DVE op — this alone frees the accumulator's
                # PSUM banks, so the next j-block's PV never waits on the
                # rest of the chain). Then 1/s = exp(-ln(s)) on ScalarE
                # (custom DVE reciprocal ops don't compile on this walrus
                # build; plain DVE reciprocal runs ~7 cycles/element;
                # AF.Reciprocal lives in a different activation-table set
                # than Exp, and a set switch is ~2.7us), broadcast by K=1
                # ones matmuls, copied to SBUF, multiplied on DVE.
                def norm_stage1(hp, j, ap):
                    cp = nrm.tile([65, 2 * QB], F32, tag="cp", name="cp")
                    nc.vector.tensor_copy(cp[:], ap[0:65, :])
                    return cp

                def norm_stage2(cp):
                    lns = nrm.tile([1, 2 * QB], F32, tag="lns", name="lns")
                    nc.scalar.activation(lns[:], cp[64:65, :], AF.Ln)
                    return lns

                def norm_stage3(cp, lns):
                    # ScalarE writes the F32R rounding the broadcast needs
                    rrr = nrm.tile([1, 2 * QB], F32R, tag="rrr", name="rrr")
                    nc.scalar.activation(rrr[:], lns[:], AF.Exp, scale=-1.0)
                    rb = nrm.tile([64, 2 * QB], F32, tag="rb", name="rb")
                    for off in (0, QB):
                        rbp = qk_ps.tile([64, QB], F32, tag="qk", name="rbp")
                        nc.tensor.matmul(
                            rbp[:],
                            onesr[:, 0:64],
                            rrr[:, off : off + QB],
                            start=True,
                            stop=True,
                        )
                        nc.vector.tensor_copy(rb[:, off : off + QB], rbp[:])
                    return rb

                def norm_stage4(hp, j, cp, rb):
                    # DVE multiply: GpSimd tensor ops measured ~35us slower
                    # per iteration on hardware than CoreSim's cost model
                    att = op.tile([64, 2 * QB], F32, tag="att_out", name="att")
                    nc.vector.tensor_mul(att[:], cp[0:64, :], rb[:])
                    hA, hB = 2 * hp, 2 * hp + 1
                    for h, off in ((hA, 0), (hB, QB)):
                        nc.sync.dma_start(
                            outT[64 * h : 64 * (h + 1), QB * j : QB * (j + 1)],
                            att[:, off : off + QB],
                        )

                norm_q = []  # (stage, hp, j, cp, aux)

                def advance_norm():
                    if not norm_q:
                        return
                    stage, hp, j, cp, aux = norm_q.pop(0)
                    if stage == 2:
                        norm_q.append((3, hp, j, cp, norm_stage2(cp)))
                    elif stage == 3:
                        norm_q.append((4, hp, j, cp, norm_stage3(cp, aux)))
                    else:
                        norm_stage4(hp, j, cp, aux)

                issue_scores(*steps[0])
                for idx, (hp, j, t) in enumerate(steps):
                    if idx + 1 < n_steps:
                        issue_scores(*steps[idx + 1])
                    # extra fillers at j starts, where the new PV accumulator
                    # waits on the previous j's normalization drain
                    boundary = t == 4 * j + 3
                    pop_filler(idx, 2 if t == 0 else 0)
                    advance_norm()
                    issue_pv(hp, j, t)
                    if boundary:
                        cp = norm_stage1(hp, j, aps_cur[0])
                        norm_q.append((2, hp, j, cp, None))
                while norm_q:
                    advance_norm()
                while filler:
                    do_item(filler.pop(0)[2])

    if split_waits:
        _split_multi_waits(nc)
    return nc


def _get_runner():
    if "nc" not in _CACHE:
        _CACHE["nc"] = build_module()
    return _CACHE["nc"]


def _make_in_maps(x, Wq, bq, Wk, bk, Wv, bv):
    x = np.asarray(x, dtype=np.float32)
    Wq = np.asarray(Wq, dtype=np.float32)
    Wk = np.asarray(Wk, dtype=np.float32)
    Wv = np.asarray(Wv, dtype=np.float32)
    bq = np.asarray(bq, dtype=np.float32)
    bk = np.asarray(bk, dtype=np.float32)
    bv = np.asarray(bv, dtype=np.float32)

    kp = np.arange(128)[:, None]
    qf = np.arange(128)[None, :]
    tri = np.where(kp <= qf, 0.0, NEG).astype(np.float32)
    trid = np.concatenate([tri, tri], axis=1)
    full = np.full((128, 128), NEG, np.float32)
    tri2 = np.concatenate([full, tri, full, tri], axis=1)
    ones = np.ones((128, 4), np.float32)

    xTs = [np.ascontiguousarray(x[b].T) for b in range(B)]
    in_maps = []
    for c in range(N_CORES):
        b = c // 4
        g = c % 4
        sl = slice(DCORE * g, DCORE * (g + 1))
        in_maps.append(
            {
                "xT": xTs[b],
                "wq": np.ascontiguousarray(Wq[:, sl]),
                "wk": np.ascontiguousarray(Wk[:, sl]),
                "wv": np.ascontiguousarray(Wv[:, sl]),
                "bq": np.ascontiguousarray(bq[sl]),
                "bk": np.ascontiguousarray(bk[sl]),
                "bv": np.ascontiguousarray(bv[sl]),
                "tri": trid,
                "tri2": tri2,
                "ones": ones,
            }
        )
    return in_maps


def kernel(x, Wq, bq, Wk, bk, Wv, bv):
    from concourse.bass_utils import run_bass_kernel_spmd

    nc = _get_runner()
    in_maps = _make_in_maps(x, Wq, bq, Wk, bk, Wv, bv)
    res = run_bass_kernel_spmd(nc, in_maps, list(range(N_CORES)))
    out = np.empty((B, S, D), dtype=np.float32)
    for c in range(N_CORES):
        b = c // 4
        g = c % 4
        out[b, :, DCORE * g : DCORE * (g + 1)] = res.results[c]["outT"].T
    return out
